# revision 1
# baseline (speedup 1.0000x reference)
"""CaptioningRNN (attention LSTM + vocab softmax loss) on 8 TRN2 NeuronCores.

Data-parallel over batch N=256 -> 32 samples/core. Weights replicated.
All matmuls bf16 (fp32 PSUM accumulate). Per-core partial losses summed on host.

Layouts (per core, B=32 samples, S=31 steps, H=1024, P16=16 spatial):
  - GEMM orientation: out = lhsT.T @ rhs with lhsT = inputT slices
    (feature-dim on partitions, batch on lhsT free), rhs = weight slices
    (feature-dim on partitions, out-cols free). Gate psum tiles (128,512)
    pack 4 units of 32 batch rows via PE col-tiling (tile_position).
  - Attention uses an all-pairs score matmul masked by a constant M32
    (block-diag mask added via an identity matmul), softmax on ACT with
    fused accumulate, and a precomputed B = A2 @ Wattn so the per-step
    attention context enters the gate GEMM as 4 extra K-chunks.
"""

import os
import numpy as np
import ml_dtypes

BF = ml_dtypes.bfloat16

N, T, V, W_DIM, H, D_IMG = 256, 32, 10000, 512, 1024, 1280
P16 = 16
NC = 8
B = N // NC          # 32 samples per core
S = T - 1            # 31 steps
ROWS = B * S         # 992 (t,n) rows per core, r = 32*t + n
MCH = 8              # vocab row chunks
MROW = ROWS // MCH   # 124
VCH = 20             # vocab col chunks
VCOL = V // VCH      # 500
NEG = -1.0e5         # mask value (exp underflows to exactly 0)

_cache = {}

last_exec_ns = None


def _build(has_b, has_bvocab, phases=3):
    import concourse.mybir as mybir
    from concourse.bacc import Bacc
    from concourse.tile import TileContext

    F32 = mybir.dt.float32
    BF16 = mybir.dt.bfloat16
    AF = mybir.ActivationFunctionType
    ALU = mybir.AluOpType
    AX = mybir.AxisListType

    nc = Bacc()

    # ---- dram parameters (per-core shapes) ----
    d_f2t = nc.declare_dram_parameter("f2t", [1408, 512], BF16, isOutput=False)
    d_wproj = nc.declare_dram_parameter("wproj", [1408, 1024], BF16, isOutput=False)
    d_wattn = nc.declare_dram_parameter("wattn", [1024, 4096], BF16, isOutput=False)
    d_wh = nc.declare_dram_parameter("wh", [1024, 4096], BF16, isOutput=False)
    d_wx = nc.declare_dram_parameter("wx", [512, 4096], BF16, isOutput=False)
    d_xt = nc.declare_dram_parameter("xt", [512, ROWS], BF16, isOutput=False)
    d_wvoc = nc.declare_dram_parameter("wvoc", [1024, V], BF16, isOutput=False)
    d_wtgt = nc.declare_dram_parameter("wtgt", [1024, ROWS], BF16, isOutput=False)
    d_maskm = nc.declare_dram_parameter("maskm", [MROW, MCH], F32, isOutput=False)
    d_i128 = nc.declare_dram_parameter("i128", [128, 128], BF16, isOutput=False)
    d_m32 = nc.declare_dram_parameter("m32", [32, 512], BF16, isOutput=False)
    if has_b:
        d_bvec = nc.declare_dram_parameter("bvec", [1, 4096], BF16, isOutput=False)
    if has_bvocab:
        d_bvoc = nc.declare_dram_parameter("bvoc", [1, V], BF16, isOutput=False)
        d_btgt = nc.declare_dram_parameter("btgt", [1, ROWS], F32, isOutput=False)
    d_loss = nc.declare_dram_parameter("loss", [1, 1], F32, isOutput=True)

    with TileContext(nc) as tc:
        with tc.tile_pool(name="ppa", bufs=1) as ppa:
            # ---- persistent tiles (live across phases) ----
            at_t = ppa.tile([128, 8, 512], BF16, tag="at")        # A2T (H-chunks, (n,p))
            bp_t = [ppa.tile([128, 4096], BF16, tag=f"bp{c}", name=f"bp{c}") for c in range(4)]
            hst_t = ppa.tile([128, 8, ROWS], BF16, tag="hst")      # hsT history
            h0t_t = ppa.tile([128, 8, B], BF16, tag="h0t")
            c_t = ppa.tile([64, 512], F32, tag="cst")              # LSTM c state
            i128_t = ppa.tile([128, 128], BF16, tag="i128")
            m32_t = ppa.tile([32, 512], BF16, tag="m32")
            ones_t = ppa.tile([1, 128], BF16, tag="ones")
            nc.sync.dma_start(i128_t[:], d_i128[:])
            nc.sync.dma_start(m32_t[:], d_m32[:])
            nc.vector.memset(ones_t[:], 1.0)
            if has_b:
                bvec_t = ppa.tile([1, 4096], BF16, tag="bvec")
                nc.sync.dma_start(bvec_t[:], d_bvec[:])

            # ================= P1: feature projection -> A2T, h0, c0 ==========
            with (
                tc.tile_pool(name="p12", bufs=1) as p12,
                tc.tile_pool(name="psa", bufs=2, space="PSUM") as psa,
            ):
                wproj_t = p12.tile([128, 11, 1024], BF16, tag="wproj")
                f2t_t = p12.tile([128, 11, 512], BF16, tag="f2t")
                nc.sync.dma_start(
                    wproj_t[:], d_wproj[:].rearrange("(c k) m -> k c m", k=128))
                nc.sync.dma_start(
                    f2t_t[:], d_f2t[:].rearrange("(c k) m -> k c m", k=128))
                for hc in range(8):
                    ps = psa.tile([128, 512], F32, tag="pp")
                    for kk in range(11):
                        nc.tensor.matmul(
                            ps[:], wproj_t[:, kk, 128 * hc:128 * (hc + 1)],
                            f2t_t[:, kk, :], start=(kk == 0), stop=(kk == 10))
                    nc.vector.tensor_copy(at_t[:, hc, :], ps[:])

                # h0 = mean over p of A ; build h0T (128,8,32) and c0 (64,512)
                h0f_t = p12.tile([128, 8, B], F32, tag="h0f")
                for hc in range(8):
                    nc.vector.reduce_sum(
                        h0f_t[:, hc, :],
                        at_t[:, hc, :].rearrange("k (n p) -> k n p", p=P16),
                        axis=AX.X)
                nc.vector.tensor_scalar(h0t_t[:], h0f_t[:],
                                        1.0 / P16, None, op0=ALU.mult)
                c0p = psa.tile([64, 512], BF16, tag="c0p")
                for k in range(8):
                    eta, j = k // 4, k % 4
                    nc.tensor.transpose(
                        c0p[32 * eta:32 * (eta + 1), 128 * j:128 * (j + 1)],
                        h0t_t[:, k, :], i128_t[:, 0:128],
                        tile_position=(0, 32 * eta))
                nc.vector.tensor_copy(c_t[:], c0p[:])

                # ================= P2: B = A2 @ Wattn ==========
                for nch in range(8):
                    wat_t = p12.tile([128, 8, 512], BF16, tag="wat")
                    nc.sync.dma_start(
                        wat_t[:],
                        d_wattn[:, 512 * nch:512 * (nch + 1)]
                        .rearrange("(c k) m -> k c m", k=128))
                    for c in range(4):
                        ps = psa.tile([128, 512], F32, tag="pp")
                        for k in range(8):
                            nc.tensor.matmul(
                                ps[:], at_t[:, k, 128 * c:128 * (c + 1)],
                                wat_t[:, k, :], start=(k == 0), stop=(k == 7))
                        nc.vector.tensor_copy(
                            bp_t[c][:, 512 * nch:512 * (nch + 1)], ps[:])

            # ================= P3: recurrence ==========
            if phases < 2:
                dbg_t = ppa.tile([1, 1], F32, tag="dbg")
                nc.vector.tensor_copy(dbg_t[:], c_t[0:1, 0:1])
                nc.sync.dma_start(d_loss[:], dbg_t[:])
            if phases >= 2:
              with (
                  tc.tile_pool(name="ppb", bufs=1) as ppb,
                  tc.tile_pool(name="ps3", bufs=2, space="PSUM") as ps3,
                  tc.tile_pool(name="ps3s", bufs=2, space="PSUM") as ps3s,
                  tc.tile_pool(name="wk3", bufs=3) as wk3,
                  tc.tile_pool(name="wk3g", bufs=1) as wk3g,
                  tc.tile_pool(name="wk3h", bufs=2) as wk3h,
              ):
                  wh_t = ppb.tile([128, 8, 4096], BF16, tag="wh")
                  wx_t = ppb.tile([128, 4, 4096], BF16, tag="wx")
                  xt_t = ppb.tile([128, 4, ROWS], BF16, tag="xt")
                  nc.sync.dma_start(
                      wh_t[:], d_wh[:].rearrange("(c k) m -> k c m", k=128))
                  nc.sync.dma_start(
                      wx_t[:], d_wx[:].rearrange("(c k) m -> k c m", k=128))
                  nc.sync.dma_start(
                      xt_t[:], d_xt[:].rearrange("(c k) m -> k c m", k=128))

                  def ht_lhs(t, k):
                      if t == 0:
                          return h0t_t[:, k, :]
                      return hst_t[:, k, B * (t - 1):B * t]

                  def emit_x(t2):
                      pA = ps3.tile([128, 512], F32, tag="pA", name=f"pA{t2}")
                      pB = ps3.tile([128, 512], F32, tag="pB", name=f"pB{t2}")
                      units2 = [(0, 0), (0, 1), (1, 0), (1, 1),
                                (2, 0), (2, 1), (3, 0), (3, 1)]
                      for u2, (g2, e2) in enumerate(units2):
                          ps2, j2 = (pA, u2) if u2 < 4 else (pB, u2 - 4)
                          lo2 = 1024 * g2 + 512 * e2
                          sl2 = slice(32 * j2, 32 * (j2 + 1))
                          for c2 in range(4):
                              nc.tensor.matmul(
                                  ps2[sl2, :], xt_t[:, c2, B * t2:B * (t2 + 1)],
                                  wx_t[:, c2, lo2:lo2 + 512],
                                  start=(c2 == 0), stop=False,
                                  tile_position=(0, 32 * j2),
                                  skip_group_check=True)
                      return pA, pB

                  ps_cur = emit_x(0)
                  for t in range(S):
                      # ---- attention scores + softmax (uses h from step t-1)
                      psS = ps3s.tile([32, 512], F32, tag="pS")
                      nc.tensor.matmul(psS[:], i128_t[0:32, 0:32], m32_t[:],
                                       start=True, stop=False)
                      for k in range(8):
                          nc.tensor.matmul(psS[:], ht_lhs(t, k), at_t[:, k, :],
                                           start=False, stop=(k == 7))
                      e_t = wk3.tile([32, 512], F32, tag="e")
                      se_t = wk3.tile([32, 1], F32, tag="se")
                      nc.scalar.activation(e_t[:], psS[:], AF.Exp,
                                           scale=float(1.0 / np.sqrt(H)),
                                           accum_out=se_t[:, 0:1])
                      re_t = wk3.tile([32, 1], F32, tag="re")
                      nc.vector.reciprocal(re_t[:], se_t[:])
                      w_t = wk3.tile([32, 512], BF16, tag="w")
                      nc.vector.tensor_scalar(w_t[:], e_t[:], re_t[:, 0:1], None,
                                              op0=ALU.mult)
                      psW = ps3s.tile([128, 8, 32], BF16, tag="pT", name="psW")[:, 0:4, :]
                      for j in range(4):
                          nc.tensor.transpose(psW[:, j, :],
                                              w_t[:, 128 * j:128 * (j + 1)],
                                              i128_t[0:32, 0:32])
                      wt_t = wk3.tile([128, 4, 32], BF16, tag="wt")
                      nc.vector.tensor_copy(wt_t[:], psW[:])

                      # ---- gate GEMM: psA=[i0,i1,f0,f1], psB=[o0,o1,g0,g1]
                      # x-MMs for this step were already emitted (pipelined,
                      # during the previous step's tail) into psA/psB.
                      psA, psB = ps_cur
                      units = [(0, 0), (0, 1), (1, 0), (1, 1),
                               (2, 0), (2, 1), (3, 0), (3, 1)]
                      for u, (g, eta) in enumerate(units):
                          ps, j = (psA, u) if u < 4 else (psB, u - 4)
                          lo = 1024 * g + 512 * eta
                          sl = slice(32 * j, 32 * (j + 1))
                          tp = (0, 32 * j)
                          for k in range(8):
                              nc.tensor.matmul(
                                  ps[sl, :], ht_lhs(t, k),
                                  wh_t[:, k, lo:lo + 512],
                                  start=False, stop=False,
                                  tile_position=tp, skip_group_check=True)
                          nb = 4 + (1 if has_b else 0)
                          for c in range(4):
                              nc.tensor.matmul(
                                  ps[sl, :], wt_t[:, c, :],
                                  bp_t[c][:, lo:lo + 512],
                                  start=False, stop=(c == nb - 1),
                                  tile_position=tp, skip_group_check=True)
                          if has_b:
                              nc.tensor.matmul(
                                  ps[sl, :], ones_t[0:1, 0:32],
                                  bvec_t[0:1, lo:lo + 512],
                                  start=False, stop=True, tile_position=tp)

                      if t + 1 < S:
                          ps_cur = emit_x(t + 1)

                      # ---- gates
                      tif_t = wk3h.tile([128, 512], F32, tag="tif")
                      nc.scalar.activation(tif_t[:], psA[:], AF.Tanh, scale=0.5)
                      to_t = wk3h.tile([64, 512], F32, tag="to")
                      nc.scalar.activation(to_t[:], psB[0:64, :], AF.Tanh, scale=0.5)
                      tg_t = wk3h.tile([64, 512], F32, tag="tg")
                      nc.scalar.activation(tg_t[:], psB[64:128, :], AF.Tanh)
                      sf_t = wk3g.tile([64, 512], F32, tag="sf")
                      nc.vector.tensor_scalar(sf_t[:], tif_t[64:128, :], 0.5, 0.5,
                                              op0=ALU.mult, op1=ALU.add)
                      si_t = wk3g.tile([64, 512], F32, tag="si")
                      nc.vector.tensor_scalar(si_t[:], tif_t[0:64, :], 0.5, 0.5,
                                              op0=ALU.mult, op1=ALU.add)
                      so_t = wk3g.tile([64, 512], F32, tag="so")
                      nc.gpsimd.tensor_scalar(so_t[:], to_t[:], 0.5, 0.5,
                                              op0=ALU.mult, op1=ALU.add)
                      u_t = wk3g.tile([64, 512], F32, tag="u")
                      nc.vector.tensor_tensor(u_t[:], sf_t[:], c_t[:], op=ALU.mult)
                      v_t = wk3g.tile([64, 512], F32, tag="v")
                      nc.gpsimd.tensor_tensor(v_t[:], si_t[:], tg_t[:], op=ALU.mult)
                      nc.vector.tensor_tensor(c_t[:], u_t[:], v_t[:], op=ALU.add)
                      tc_t = wk3h.tile([64, 512], F32, tag="tc")
                      nc.scalar.activation(tc_t[:], c_t[:], AF.Tanh)
                      h0_t = wk3.tile([32, 512], BF16, tag="h0")
                      h1_t = wk3.tile([32, 512], BF16, tag="h1")
                      nc.vector.tensor_tensor(h0_t[:], so_t[0:32, :], tc_t[0:32, :],
                                              op=ALU.mult)
                      nc.gpsimd.tensor_tensor(h1_t[:], so_t[32:64, :], tc_t[32:64, :],
                                              op=ALU.mult)
                      psH = ps3s.tile([128, 8, 32], BF16, tag="pT")
                      for k in range(8):
                          src = h1_t if k >= 4 else h0_t
                          j = k % 4
                          nc.tensor.transpose(psH[:, k, :],
                                              src[:, 128 * j:128 * (j + 1)],
                                              i128_t[0:32, 0:32])
                      nc.vector.tensor_copy(
                          hst_t[:, :, B * t:B * (t + 1)], psH[:])

            # ================= P4: vocab scores -> loss ==========
            if phases == 2:
                dbg2_t = ppa.tile([1, 1], F32, tag="dbg2")
                nc.vector.tensor_copy(dbg2_t[:], c_t[0:1, 0:1])
                nc.sync.dma_start(d_loss[:], dbg2_t[:])
            if phases >= 3:
              with (
                  tc.tile_pool(name="p4", bufs=1) as p4,
                  tc.tile_pool(name="wk4", bufs=3) as wk4,
                  tc.tile_pool(name="ps4", bufs=4, space="PSUM") as ps4,
              ):
                  se_t = p4.tile([MROW, MCH, VCH], F32, tag="SE")
                  for vc in range(VCH):
                      wv_t = wk4.tile([128, 8, VCOL], BF16, tag="wv")
                      nc.sync.dma_start(
                          wv_t[:],
                          d_wvoc[:, VCOL * vc:VCOL * (vc + 1)]
                          .rearrange("(c k) m -> k c m", k=128))
                      if has_bvocab:
                          bvoc_t = wk4.tile([1, VCOL], BF16, tag="bvoc")
                          nc.sync.dma_start(
                              bvoc_t[:], d_bvoc[:, VCOL * vc:VCOL * (vc + 1)])
                      for m in range(MCH):
                          ps = ps4.tile([MROW, VCOL], F32, tag="pv")
                          nk = 8 + (1 if has_bvocab else 0)
                          for k in range(8):
                              nc.tensor.matmul(
                                  ps[:], hst_t[:, k, MROW * m:MROW * (m + 1)],
                                  wv_t[:, k, :], start=(k == 0), stop=(k == nk - 1))
                          if has_bvocab:
                              nc.tensor.matmul(
                                  ps[:], ones_t[0:1, 0:MROW], bvoc_t[0:1, :],
                                  start=False, stop=True)
                          scr = wk4.tile([MROW, VCOL], F32, tag="scr")
                          nc.scalar.activation(scr[:], ps[:], AF.Exp,
                                               accum_out=se_t[:, m, vc:vc + 1])

                  # target scores: sum over all rows of hsT*WtgtT (mask folded in)
                  wtgt_t = p4.tile([128, 8, ROWS], BF16, tag="wtgt")
                  nc.sync.dma_start(
                      wtgt_t[:], d_wtgt[:].rearrange("(c k) m -> k c m", k=128))
                  tparts = p4.tile([128, 8], F32, tag="tparts")
                  for k in range(8):
                      scr2 = wk4.tile([128, ROWS], F32, tag="scr2")
                      nc.vector.tensor_tensor(scr2[:], hst_t[:, k, :],
                                              wtgt_t[:, k, :], op=ALU.mult)
                      nc.vector.reduce_sum(tparts[:, k:k + 1], scr2[:],
                                           axis=AX.X)
                  tacc = p4.tile([128, 1], F32, tag="tacc")
                  nc.vector.reduce_sum(tacc[:], tparts[:], axis=AX.X)
                  tgt_r = p4.tile([128, 1], F32, tag="tgtr")
                  import concourse.bass_isa as bass_isa
                  nc.gpsimd.partition_all_reduce(tgt_r[:], tacc[:], channels=128,
                                                 reduce_op=bass_isa.ReduceOp.add)

                  # lse side
                  ses_t = p4.tile([MROW, MCH], F32, tag="ses")
                  nc.vector.reduce_sum(ses_t[:], se_t[:], axis=AX.X)
                  l_t = p4.tile([MROW, MCH], F32, tag="lt")
                  nc.scalar.activation(l_t[:], ses_t[:], AF.Ln)
                  maskm_t = p4.tile([MROW, MCH], F32, tag="maskm")
                  nc.sync.dma_start(maskm_t[:], d_maskm[:])
                  lm_t = p4.tile([MROW, MCH], F32, tag="lm")
                  nc.vector.tensor_tensor(lm_t[:], l_t[:], maskm_t[:], op=ALU.mult)
                  lr_t = p4.tile([MROW, 1], F32, tag="lr")
                  nc.vector.reduce_sum(lr_t[:], lm_t[:], axis=AX.X)
                  lse_r = p4.tile([MROW, 1], F32, tag="lser")
                  nc.gpsimd.partition_all_reduce(lse_r[:], lr_t[:], channels=MROW,
                                                 reduce_op=bass_isa.ReduceOp.add)

                  nll_t = p4.tile([1, 1], F32, tag="nll")
                  nc.vector.tensor_tensor(nll_t[:], lse_r[0:1, :], tgt_r[0:1, :],
                                          op=ALU.subtract)
                  if has_bvocab:
                      btgt_t = p4.tile([1, ROWS], F32, tag="btgt")
                      nc.sync.dma_start(btgt_t[:], d_btgt[:])
                      bts_t = p4.tile([1, 1], F32, tag="bts")
                      nc.vector.reduce_sum(bts_t[:], btgt_t[:], axis=AX.X)
                      nc.vector.tensor_tensor(nll_t[:], nll_t[:], bts_t[:],
                                              op=ALU.subtract)
                  loss_t = p4.tile([1, 1], F32, tag="loss")
                  nc.vector.tensor_scalar(loss_t[:], nll_t[:], 1.0 / N, None,
                                          op0=ALU.mult)
                  nc.sync.dma_start(d_loss[:], loss_t[:])

    nc.finalize()
    return nc


def kernel(features, captions, W_proj, b_proj, W_embed, Wx, Wh, Wattn, b,
           W_vocab, b_vocab):
    global last_exec_ns
    from concourse.bass_utils import run_bass_kernel_spmd

    features = np.asarray(features)
    captions = np.asarray(captions)
    cap_dtype = captions.dtype
    W_proj = np.asarray(W_proj, np.float32)
    b_proj = np.asarray(b_proj, np.float32)
    W_embed = np.asarray(W_embed, np.float32)
    Wx = np.asarray(Wx, np.float32)
    Wh = np.asarray(Wh, np.float32)
    Wattn = np.asarray(Wattn, np.float32)
    b = np.asarray(b, np.float32)
    W_vocab = np.asarray(W_vocab, np.float32)
    b_vocab = np.asarray(b_vocab, np.float32)

    has_b = bool(np.any(b))
    has_bvocab = bool(np.any(b_vocab))

    phases = int(os.environ.get("BASS_PHASES", "3"))
    key = (has_b, has_bvocab, phases)
    if key not in _cache:
        _cache[key] = _build(has_b, has_bvocab, phases)
    nc = _cache[key]

    cap_in = np.asarray(captions[:, :-1], np.int64)   # (N, S)
    cap_out = np.asarray(captions[:, 1:], np.int64)
    mask = (cap_out != 0).astype(np.float32)          # (N, S)
    x = W_embed[cap_in].astype(np.float32)            # (N, S, W_DIM)

    # shared (replicated) arrays
    wproj_h = np.zeros((1408, 1024), np.float32)
    wproj_h[:D_IMG] = W_proj
    wproj_h[D_IMG] = b_proj
    wproj_h = wproj_h.astype(BF)
    wh_h = Wh.astype(BF)
    wx_h = Wx.astype(BF)
    wattn_h = Wattn.astype(BF)
    wvoc_h = W_vocab.astype(BF)
    i128_h = np.eye(128, dtype=BF)
    col_n = np.arange(B * P16) // P16
    m32_h = np.where(col_n[None, :] == np.arange(B)[:, None], 0.0, NEG
                     ).astype(BF)
    bvec_h = b.reshape(1, 4096).astype(BF)
    bvoc_h = b_vocab.reshape(1, V).astype(BF)

    feat = features.reshape(N, D_IMG, P16).astype(np.float32)

    in_maps = []
    for ci in range(NC):
        sl = slice(ci * B, (ci + 1) * B)
        f2t = np.zeros((1408, 512), np.float32)
        f2t[:D_IMG] = feat[sl].transpose(1, 0, 2).reshape(D_IMG, B * P16)
        f2t[D_IMG] = 1.0
        xt = x[sl].transpose(2, 1, 0).reshape(W_DIM, ROWS)  # col = 32*t + n
        tgt = cap_out[sl].T.reshape(ROWS)                   # r = 32*t + n
        mk = mask[sl].T.reshape(ROWS)
        wtgt = (W_vocab[:, tgt] * mk[None, :]).astype(BF)
        maskm = mk.reshape(MCH, MROW).T.copy()              # [row, m]
        m = {
            "f2t": f2t.astype(BF),
            "wproj": wproj_h,
            "wattn": wattn_h,
            "wh": wh_h,
            "wx": wx_h,
            "xt": xt.astype(BF),
            "wvoc": wvoc_h,
            "wtgt": wtgt,
            "maskm": maskm.astype(np.float32),
            "i128": i128_h,
            "m32": m32_h,
        }
        if has_b:
            m["bvec"] = bvec_h
        if has_bvocab:
            m["bvoc"] = bvoc_h
            m["btgt"] = (b_vocab[tgt] * mk).reshape(1, ROWS).astype(np.float32)
        in_maps.append(m)

    trace = bool(int(os.environ.get("BASS_KPROF", "0")))
    if trace:
        import sys, types
        try:
            import antenv.axon_hooks  # noqa
        except ImportError:
            import trn_agent_boot.trn_boot as _tb
            _hook = _tb._ntff_profile_via_ctypes("/opt/axon/libaxon_pjrt.so")
            _mod = types.ModuleType("antenv.axon_hooks")
            _mod.get_axon_ntff_profile_hook = lambda: _hook
            import antenv
            sys.modules["antenv.axon_hooks"] = _mod
            antenv.axon_hooks = _mod

    if os.environ.get("BASS_SIM"):
        from concourse.bass_interp import CoreSim
        sim = CoreSim(nc)
        for k2, v2 in in_maps[0].items():
            sim.tensor(k2)[:] = v2
        sim.simulate()
        print("SIM core0 partial loss:", np.asarray(sim.tensor("loss"))[0, 0],
              flush=True)
        return np.asarray(np.float32(np.asarray(sim.tensor("loss"))[0, 0] * NC))

    res = run_bass_kernel_spmd(nc, in_maps, core_ids=list(range(NC)),
                               trace=trace)
    last_exec_ns = res.exec_time_ns
    total = np.float32(0.0)
    for ci in range(NC):
        total += res.results[ci]["loss"][0, 0]
    out = np.asarray(total, np.float32)
    del cap_dtype
    return out



# revision 8
# speedup vs baseline: 1.9915x; 1.9915x over previous
"""CaptioningRNN (attention LSTM + vocab softmax loss) on 8 TRN2 NeuronCores.

Data-parallel over batch N=256 -> 32 samples/core. Weights replicated.

Key performance structure (vs naive):
  - Recurrence matmuls are emitted K-chunk-outer / unit-interleaved so that
    consecutive MMs target different PE column tiles (tile_position
    (0,32j)) and stream concurrently (~4x effective MM rate).
  - Gate GEMM per step is phased so tensor work overlaps the softmax and
    LSTM activation chains: [scores || h-part j1-3], [h-part j0 || h-part B],
    [attn], [x(t+1) during activation chain].
  - P2 (A2 @ Wattn) and P4 (vocab scores) run in fp8(e4m3) DoubleRow mode:
    half the MMs at same N. Operands are pre-scaled (x16 activations,
    x512 weights) and the 1/8192 product scale is folded into the psum
    consumer (copy-scale or exp-scale).
  - The target-score path (subtracted from LSE) stays bf16 exact.
  - wh/wx/xt DMAs are issued up front and stream during P1/P2 compute.
"""

import math
import os
import numpy as np
import ml_dtypes

BF = ml_dtypes.bfloat16
FP8NP = ml_dtypes.float8_e4m3

N, T, V, W_DIM, H, D_IMG = 256, 32, 10000, 512, 1024, 1280
P16 = 16
NC = 8
B = N // NC          # 32 samples per core
S = T - 1            # 31 steps
ROWS = B * S         # 992 (t,n) rows per core, r = 32*t + n
MCH = 8              # vocab row chunks
MROW = ROWS // MCH   # 124
VCH = 20             # vocab col chunks
VCOL = V // VCH      # 500
NEG = -1.0e5         # mask value (exp underflows to exactly 0)
FSCALE = 16.0        # activation fp8 scale
WSCALE = 512.0       # weight fp8 scale
PSCALE = FSCALE * WSCALE  # product scale in psum

_cache = {}

last_exec_ns = None


def _build(has_b, has_bvocab, phases=3):
    import concourse.mybir as mybir
    from concourse.bacc import Bacc
    from concourse.tile import TileContext

    F32 = mybir.dt.float32
    BF16 = mybir.dt.bfloat16
    FP8 = mybir.dt.float8e4
    AF = mybir.ActivationFunctionType
    ALU = mybir.AluOpType
    AX = mybir.AxisListType
    DR = mybir.MatmulPerfMode.DoubleRow

    nc = Bacc()

    # ---- dram parameters (per-core shapes) ----
    d_f2t = nc.declare_dram_parameter("f2t", [1408, 512], BF16, isOutput=False)
    d_wproj = nc.declare_dram_parameter("wproj", [1408, 1024], BF16, isOutput=False)
    d_wattn = nc.declare_dram_parameter("wattn", [1024, 4096], FP8, isOutput=False)
    d_wh = nc.declare_dram_parameter("wh", [1024, 4096], BF16, isOutput=False)
    d_wx = nc.declare_dram_parameter("wx", [512, 4096], BF16, isOutput=False)
    d_xt = nc.declare_dram_parameter("xt", [512, ROWS], BF16, isOutput=False)
    d_wvoc = nc.declare_dram_parameter("wvoc", [1024, V], FP8, isOutput=False)
    d_wtgt = nc.declare_dram_parameter("wtgt", [1024, ROWS], BF16, isOutput=False)
    d_maskm = nc.declare_dram_parameter("maskm", [MROW, MCH], F32, isOutput=False)
    d_i128 = nc.declare_dram_parameter("i128", [128, 128], BF16, isOutput=False)
    d_m32 = nc.declare_dram_parameter("m32", [32, 512], BF16, isOutput=False)
    if has_b:
        d_bvec = nc.declare_dram_parameter("bvec", [1, 4096], BF16, isOutput=False)
    if has_bvocab:
        # pre-scaled by PSCALE on host to match the fp8 psum scale
        d_bvoc = nc.declare_dram_parameter("bvoc", [1, V], BF16, isOutput=False)
        d_btgt = nc.declare_dram_parameter("btgt", [1, ROWS], F32, isOutput=False)
    d_loss = nc.declare_dram_parameter("loss", [1, 1], F32, isOutput=True)

    # gate-unit tables: psA = [i0,i1,f0,f1], psB = [o0,o1,g0,g1]
    UNITS_A = [(0, 0), (0, 1), (1, 0), (1, 1)]
    UNITS_B = [(2, 0), (2, 1), (3, 0), (3, 1)]

    def loA(j):
        g, e = UNITS_A[j]
        return 1024 * g + 512 * e

    def loB(j):
        g, e = UNITS_B[j]
        return 1024 * g + 512 * e

    def sl(j):
        return slice(32 * j, 32 * (j + 1))

    with TileContext(nc) as tc:
        ppa = tc.alloc_tile_pool(name="ppa", bufs=1)
        i128_t = ppa.tile([128, 128], BF16, tag="i128")
        nc.sync.dma_start(i128_t[:], d_i128[:])
        if has_b or has_bvocab:
            ones_t = ppa.tile([1, 128], BF16, tag="ones")
            nc.vector.memset(ones_t[:], 1.0)

        # pools alive P1..P3
        pp3 = tc.alloc_tile_pool(name="pp3", bufs=1)
        ppb = tc.alloc_tile_pool(name="ppb", bufs=1)
        at_t = pp3.tile([128, 8, 512], BF16, tag="at")         # A2T
        bp_t = [pp3.tile([128, 4096], BF16, tag=f"bp{c}", name=f"bp{c}")
                for c in range(4)]
        h0t_t = pp3.tile([128, 8, B], BF16, tag="h0t")
        c_t = pp3.tile([64, 512], F32, tag="cst")
        m32_t = pp3.tile([32, 512], BF16, tag="m32")
        wh_t = ppb.tile([128, 8, 4096], BF16, tag="wh")
        wx_t = ppb.tile([128, 4, 4096], BF16, tag="wx")
        xt_t = ppb.tile([128, 4, ROWS], BF16, tag="xt")

        # P1/P2 scratch
        p1 = tc.alloc_tile_pool(name="p1", bufs=1)
        p2 = tc.alloc_tile_pool(name="p2", bufs=1)
        p1ps = tc.alloc_tile_pool(name="p1ps", bufs=2, space="PSUM")
        p2ps = tc.alloc_tile_pool(name="p2ps", bufs=2, space="PSUM")

        wproj_t = p1.tile([128, 11, 1024], BF16, tag="wproj")
        f2t_t = p1.tile([128, 11, 512], BF16, tag="f2t")
        at8_t = p2.tile([128, 8, 512], FP8, tag="at8")
        # wattn fp8, 4 chunks of 1024 gate-cols, double buffered
        wat_t = [p2.tile([128, 8, 1024], FP8, tag=f"wat{i}", name=f"wat{i}")
                 for i in range(2)]

        # ---- DMA issue order (priority): P1 deps, m32, wattn, xt, wh, wx ----
        nc.sync.dma_start(
            wproj_t[:], d_wproj[:].rearrange("(c k) m -> k c m", k=128))
        nc.sync.dma_start(
            f2t_t[:], d_f2t[:].rearrange("(c k) m -> k c m", k=128))
        nc.sync.dma_start(m32_t[:], d_m32[:])
        nc.sync.dma_start(
            wat_t[0][:],
            d_wattn[:, 0:1024].rearrange("(c k) m -> k c m", k=128))
        nc.sync.dma_start(
            wat_t[1][:],
            d_wattn[:, 1024:2048].rearrange("(c k) m -> k c m", k=128))
        nc.sync.dma_start(
            xt_t[:], d_xt[:].rearrange("(c k) m -> k c m", k=128))
        nc.sync.dma_start(
            wh_t[:], d_wh[:].rearrange("(c k) m -> k c m", k=128))
        nc.sync.dma_start(
            wx_t[:], d_wx[:].rearrange("(c k) m -> k c m", k=128))
        if has_b:
            bvec_t = pp3.tile([1, 4096], BF16, tag="bvec")
            nc.sync.dma_start(bvec_t[:], d_bvec[:])

        # ================= P1: feature projection -> A2T, h0, c0 ==========
        for hc in range(8):
            ps = p1ps.tile([128, 512], F32, tag="pp1")
            for kk in range(11):
                nc.tensor.matmul(
                    ps[:], wproj_t[:, kk, 128 * hc:128 * (hc + 1)],
                    f2t_t[:, kk, :], start=(kk == 0), stop=(kk == 10))
            nc.vector.tensor_copy(at_t[:, hc, :], ps[:])
            nc.vector.tensor_scalar(at8_t[:, hc, :], ps[:],
                                    FSCALE, None, op0=ALU.mult)

        # h0 = mean over p of A ; h0T (128,8,32) and c0 (64,512)
        h0f_t = p1.tile([128, 8, B], F32, tag="h0f")
        for hc in range(8):
            nc.vector.reduce_sum(
                h0f_t[:, hc, :],
                at_t[:, hc, :].rearrange("k (n p) -> k n p", p=P16),
                axis=AX.X)
        nc.vector.tensor_scalar(h0t_t[:], h0f_t[:],
                                1.0 / P16, None, op0=ALU.mult)
        c0p = p2ps.tile([64, 512], BF16, tag="c0p")
        for k in range(8):
            eta, j = k // 4, k % 4
            nc.tensor.transpose(
                c0p[32 * eta:32 * (eta + 1), 128 * j:128 * (j + 1)],
                h0t_t[:, k, :], i128_t[0:128, 0:128],
                tile_position=(0, 32 * eta))
        nc.vector.tensor_copy(c_t[:], c0p[:])

        # ================= P2: B = A2 @ Wattn (fp8 DoubleRow) ==========
        for half in range(4):  # 1024 gate-cols each
            wt_buf = wat_t[half % 2]
            if half >= 2:
                nc.sync.dma_start(
                    wt_buf[:],
                    d_wattn[:, 1024 * half:1024 * (half + 1)]
                    .rearrange("(c k) m -> k c m", k=128))
            for nch in range(2):  # 512-col chunks within the half
                for c in range(4):
                    ps = p2ps.tile([128, 512], F32, tag="pp2")
                    for kc in range(4):
                        nc.tensor.matmul(
                            ps[:], at8_t[:, 2 * kc:2 * kc + 2,
                                         128 * c:128 * (c + 1)],
                            wt_buf[:, 2 * kc:2 * kc + 2,
                                   512 * nch:512 * (nch + 1)],
                            start=(kc == 0), stop=(kc == 3),
                            perf_mode=DR)
                    nc.vector.tensor_scalar(
                        bp_t[c][:, 1024 * half + 512 * nch:
                                1024 * half + 512 * (nch + 1)],
                        ps[:], 1.0 / PSCALE, None, op0=ALU.mult)

        p2ps.release()
        p1ps.release()
        p2.release()
        p1.release()

        if phases < 2:
            dbg_t = ppa.tile([1, 1], F32, tag="dbg")
            nc.vector.tensor_copy(dbg_t[:], c_t[0:1, 0:1])
            nc.sync.dma_start(d_loss[:], dbg_t[:])

        # pools alive P3..P4
        pph = tc.alloc_tile_pool(name="pph", bufs=1, side="right")
        hst_t = pph.tile([128, 8, ROWS], BF16, tag="hst")
        hst8_t = pph.tile([128, 8, ROWS], FP8, tag="hst8")
        p4io = tc.alloc_tile_pool(name="p4io", bufs=3, side="right")
        wv_pre = {}
        for vc in range(2):
            wv = p4io.tile([128, 8, VCOL], FP8, tag="wv", name=f"wv{vc}")
            nc.sync.dma_start(
                wv[:], d_wvoc[:, VCOL * vc:VCOL * (vc + 1)]
                .rearrange("(c k) m -> k c m", k=128))
            wv_pre[vc] = wv

        # ================= P3: recurrence ==========
        if phases >= 2:
            ps3 = tc.alloc_tile_pool(name="ps3", bufs=2, space="PSUM")
            psSp = tc.alloc_tile_pool(name="psSp", bufs=2, space="PSUM")
            psTp = tc.alloc_tile_pool(name="psTp", bufs=2, space="PSUM")
            wkS = tc.alloc_tile_pool(name="wkS", bufs=2)
            wkC = tc.alloc_tile_pool(name="wkC", bufs=1)

            def ht(t, k):
                if t == 0:
                    return h0t_t[:, k, :]
                return hst_t[:, k, B * (t - 1):B * t]

            def mm(ps, lhs, rhs, start, stop, tp):
                nc.tensor.matmul(ps, lhs, rhs, start=start, stop=stop,
                                 tile_position=tp, skip_group_check=True)

            def emit_x0():
                pA = ps3.tile([128, 512], F32, tag="pA", name="pA0")
                pB = ps3.tile([128, 512], F32, tag="pB", name="pB0")
                for c in range(4):
                    for j in range(4):
                        mm(pA[sl(j), :], xt_t[:, c, 0:B],
                           wx_t[:, c, loA(j):loA(j) + 512],
                           start=(c == 0), stop=False, tp=(0, 32 * j))
                    for j in range(4):
                        mm(pB[sl(j), :], xt_t[:, c, 0:B],
                           wx_t[:, c, loB(j):loB(j) + 512],
                           start=(c == 0), stop=False, tp=(0, 32 * j))
                return pA, pB

            ps_cur = emit_x0()
            inv_sqrt_h = float(1.0 / np.sqrt(H))

            for t in range(S):
                psA, psB = ps_cur
                # --- scores (tile 0) interleaved with h-part of psA j1..3
                psS = psSp.tile([32, 512], F32, tag="pS")
                nc.tensor.matmul(psS[:], i128_t[0:32, 0:32], m32_t[:],
                                 start=True, stop=False)
                for k in range(8):
                    nc.tensor.matmul(psS[:], ht(t, k), at_t[:, k, :],
                                     start=False, stop=(k == 7))
                    for j in (1, 2, 3):
                        mm(psA[sl(j), :], ht(t, k),
                           wh_t[:, k, loA(j):loA(j) + 512],
                           start=False, stop=False, tp=(0, 32 * j))
                # softmax (scalar + vector)
                e_t = wkS.tile([32, 512], F32, tag="e")
                se_t = wkS.tile([32, 1], F32, tag="se")
                nc.scalar.activation(e_t[:], psS[:], AF.Exp,
                                     scale=inv_sqrt_h, accum_out=se_t[:, 0:1])
                re_t = wkS.tile([32, 1], F32, tag="re")
                nc.vector.reciprocal(re_t[:], se_t[:])
                w_t = wkS.tile([32, 512], BF16, tag="w")
                nc.vector.tensor_scalar(w_t[:], e_t[:], re_t[:, 0:1], None,
                                        op0=ALU.mult)
                # --- h-part psA j0 + psB j1..3
                for k in range(8):
                    mm(psA[sl(0), :], ht(t, k),
                       wh_t[:, k, loA(0):loA(0) + 512],
                       start=False, stop=False, tp=(0, 0))
                    for j in (1, 2, 3):
                        mm(psB[sl(j), :], ht(t, k),
                           wh_t[:, k, loB(j):loB(j) + 512],
                           start=False, stop=False, tp=(0, 32 * j))
                # --- w transposes (wait softmax)
                psW = psTp.tile([128, 4, 64], BF16, tag="pT",
                                name=f"psW{t}")[:, :, 0:32]
                for c in range(4):
                    nc.tensor.transpose(psW[:, c, :],
                                        w_t[:, 128 * c:128 * (c + 1)],
                                        i128_t[0:32, 0:32])
                wt_t = wkS.tile([128, 4, 32], BF16, tag="wt")
                nc.vector.tensor_copy(wt_t[:], psW[:])
                # --- attn psA all units (4-way)
                for c in range(4):
                    for j in range(4):
                        mm(psA[sl(j), :], wt_t[:, c, :],
                           bp_t[c][:, loA(j):loA(j) + 512],
                           start=False, stop=(c == 3 and not has_b),
                           tp=(0, 32 * j))
                if has_b:
                    for j in range(4):
                        nc.tensor.matmul(
                            psA[sl(j), :], ones_t[0:1, 0:32],
                            bvec_t[0:1, loA(j):loA(j) + 512],
                            start=False, stop=True, tile_position=(0, 32 * j),
                            skip_group_check=True)
                # sigma(i), sigma(f) + u = f*c
                si_t = wkC.tile([64, 512], F32, tag="si")
                nc.scalar.activation(si_t[:], psA[0:64, :], AF.Sigmoid)
                sf_t = wkC.tile([64, 512], F32, tag="sf")
                nc.scalar.activation(sf_t[:], psA[64:128, :], AF.Sigmoid)
                u_t = wkC.tile([64, 512], F32, tag="u")
                nc.vector.tensor_tensor(u_t[:], sf_t[:], c_t[:],
                                        op=ALU.mult)
                # --- h-part psB j0 + attn psB j1..3
                for k in range(8):
                    mm(psB[sl(0), :], ht(t, k),
                       wh_t[:, k, loB(0):loB(0) + 512],
                       start=False, stop=False, tp=(0, 0))
                    if k < 4:
                        for j in (1, 2, 3):
                            mm(psB[sl(j), :], wt_t[:, k, :],
                               bp_t[k][:, loB(j):loB(j) + 512],
                               start=False, stop=(k == 3 and not has_b),
                               tp=(0, 32 * j))
                # --- attn psB j0 + x(t+1) psA j1..3
                if t + 1 < S:
                    pA2 = ps3.tile([128, 512], F32, tag="pA", name=f"pA{t+1}")
                    pB2 = ps3.tile([128, 512], F32, tag="pB", name=f"pB{t+1}")
                for c in range(4):
                    mm(psB[sl(0), :], wt_t[:, c, :],
                       bp_t[c][:, loB(0):loB(0) + 512],
                       start=False, stop=(c == 3 and not has_b), tp=(0, 0))
                    if t + 1 < S:
                        for j in (1, 2, 3):
                            mm(pA2[sl(j), :], xt_t[:, c, B * (t + 1):B * (t + 2)],
                               wx_t[:, c, loA(j):loA(j) + 512],
                               start=(c == 0), stop=False, tp=(0, 32 * j))
                if has_b:
                    for j in range(4):
                        nc.tensor.matmul(
                            psB[sl(j), :], ones_t[0:1, 0:32],
                            bvec_t[0:1, loB(j):loB(j) + 512],
                            start=False, stop=True, tile_position=(0, 32 * j),
                            skip_group_check=True)
                # --- gate chain (scalar/vector) + x(t+1) rest on tensor
                tg_t = wkC.tile([64, 512], F32, tag="tg")
                nc.scalar.activation(tg_t[:], psB[64:128, :], AF.Tanh)
                so_t = wkC.tile([64, 512], F32, tag="so")
                nc.scalar.activation(so_t[:], psB[0:64, :], AF.Sigmoid)
                if t + 1 < S:
                    for c in range(4):
                        mm(pA2[sl(0), :], xt_t[:, c, B * (t + 1):B * (t + 2)],
                           wx_t[:, c, loA(0):loA(0) + 512],
                           start=(c == 0), stop=False, tp=(0, 0))
                        for j in (1, 2, 3):
                            mm(pB2[sl(j), :], xt_t[:, c, B * (t + 1):B * (t + 2)],
                               wx_t[:, c, loB(j):loB(j) + 512],
                               start=(c == 0), stop=False, tp=(0, 32 * j))
                    for c in range(4):
                        mm(pB2[sl(0), :], xt_t[:, c, B * (t + 1):B * (t + 2)],
                           wx_t[:, c, loB(0):loB(0) + 512],
                           start=(c == 0), stop=False, tp=(0, 0))
                    ps_cur = (pA2, pB2)
                v_t = wkC.tile([64, 512], F32, tag="v")
                nc.vector.tensor_tensor(v_t[:], si_t[:], tg_t[:],
                                        op=ALU.mult)
                nc.vector.tensor_tensor(c_t[:], u_t[:], v_t[:], op=ALU.add)
                tc_t = wkC.tile([64, 512], F32, tag="tc")
                nc.scalar.activation(tc_t[:], c_t[:], AF.Tanh)
                h_t = wkC.tile([64, 512], BF16, tag="h")
                nc.vector.tensor_tensor(h_t[:], so_t[:], tc_t[:], op=ALU.mult)
                # --- transpose h -> hst (+ fp8 copy for P4)
                psH = psTp.tile([128, 4, 64], BF16, tag="pT",
                                name=f"psH{t}")
                for q in range(4):
                    nc.tensor.transpose(psH[:, q, :],
                                        h_t[:, 128 * q:128 * (q + 1)],
                                        i128_t[0:64, 0:64])
                dst = hst_t[:, :, B * t:B * (t + 1)].rearrange(
                    "k (eta hcp) n -> k hcp eta n", eta=2, hcp=4)
                nc.vector.tensor_copy(
                    dst, psH[:].rearrange("k hcp (eta n) -> k hcp eta n", eta=2))
                nc.vector.tensor_scalar(
                    hst8_t[:, :, B * t:B * (t + 1)],
                    hst_t[:, :, B * t:B * (t + 1)],
                    FSCALE, None, op0=ALU.mult)

            wkC.release()
            wkS.release()
            psTp.release()
            psSp.release()
            ps3.release()

        ppb.release()
        pp3.release()

        if phases == 2:
            dbg2_t = ppa.tile([1, 1], F32, tag="dbg2")
            nc.vector.tensor_copy(dbg2_t[:], c_t[0:1, 0:1])
            nc.sync.dma_start(d_loss[:], dbg2_t[:])

        # ================= P4: vocab scores -> loss (fp8 DoubleRow) =======
        if phases >= 3:
            p4 = tc.alloc_tile_pool(name="p4", bufs=1, side="right")
            wk4 = tc.alloc_tile_pool(name="wk4", bufs=3, side="right")
            ps4 = tc.alloc_tile_pool(name="ps4", bufs=4, space="PSUM")

            se_t = p4.tile([MROW, MCH, VCH], F32, tag="SE")
            for vc in range(VCH):
                if vc in wv_pre:
                    wv_t = wv_pre[vc]
                else:
                    wv_t = p4io.tile([128, 8, VCOL], FP8, tag="wv",
                                     name=f"wv{vc}")
                    nc.sync.dma_start(
                        wv_t[:],
                        d_wvoc[:, VCOL * vc:VCOL * (vc + 1)]
                        .rearrange("(c k) m -> k c m", k=128))
                if has_bvocab:
                    bvoc_t = wk4.tile([1, VCOL], BF16, tag="bvoc")
                    nc.sync.dma_start(
                        bvoc_t[:], d_bvoc[:, VCOL * vc:VCOL * (vc + 1)])
                for m in range(MCH):
                    ps = ps4.tile([MROW, VCOL], F32, tag="pv")
                    for kc in range(4):
                        nc.tensor.matmul(
                            ps[:],
                            hst8_t[:, 2 * kc:2 * kc + 2,
                                   MROW * m:MROW * (m + 1)],
                            wv_t[:, 2 * kc:2 * kc + 2, :],
                            start=(kc == 0),
                            stop=(kc == 3 and not has_bvocab),
                            perf_mode=DR)
                    if has_bvocab:
                        nc.tensor.matmul(
                            ps[:], ones_t[0:1, 0:MROW], bvoc_t[0:1, :],
                            start=False, stop=True)
                    scr = wk4.tile([MROW, VCOL], F32, tag="scr")
                    nc.scalar.activation(scr[:], ps[:], AF.Exp,
                                         scale=float(1.0 / PSCALE),
                                         accum_out=se_t[:, m, vc:vc + 1])

            # target scores: sum over all rows of hsT*WtgtT (mask folded in)
            wtgt_t = p4.tile([128, 8, ROWS], BF16, tag="wtgt")
            nc.sync.dma_start(
                wtgt_t[:], d_wtgt[:].rearrange("(c k) m -> k c m", k=128))
            tparts = p4.tile([128, 8], F32, tag="tparts")
            for k in range(8):
                scr2 = wk4.tile([128, ROWS], F32, tag="scr2")
                nc.vector.tensor_tensor(scr2[:], hst_t[:, k, :],
                                        wtgt_t[:, k, :], op=ALU.mult)
                nc.vector.reduce_sum(tparts[:, k:k + 1], scr2[:],
                                     axis=AX.X)
            tacc = p4.tile([128, 1], F32, tag="tacc")
            nc.vector.reduce_sum(tacc[:], tparts[:], axis=AX.X)
            tgt_r = p4.tile([128, 1], F32, tag="tgtr")
            import concourse.bass_isa as bass_isa
            nc.gpsimd.partition_all_reduce(tgt_r[:], tacc[:], channels=128,
                                           reduce_op=bass_isa.ReduceOp.add)

            # lse side
            ses_t = p4.tile([MROW, MCH], F32, tag="ses")
            nc.vector.reduce_sum(ses_t[:], se_t[:], axis=AX.X)
            l_t = p4.tile([MROW, MCH], F32, tag="lt")
            nc.scalar.activation(l_t[:], ses_t[:], AF.Ln)
            maskm_t = p4.tile([MROW, MCH], F32, tag="maskm")
            nc.sync.dma_start(maskm_t[:], d_maskm[:])
            lm_t = p4.tile([MROW, MCH], F32, tag="lm")
            nc.vector.tensor_tensor(lm_t[:], l_t[:], maskm_t[:], op=ALU.mult)
            lr_t = p4.tile([MROW, 1], F32, tag="lr")
            nc.vector.reduce_sum(lr_t[:], lm_t[:], axis=AX.X)
            lse_r = p4.tile([MROW, 1], F32, tag="lser")
            nc.gpsimd.partition_all_reduce(lse_r[:], lr_t[:], channels=MROW,
                                           reduce_op=bass_isa.ReduceOp.add)

            nll_t = p4.tile([1, 1], F32, tag="nll")
            nc.vector.tensor_tensor(nll_t[:], lse_r[0:1, :], tgt_r[0:1, :],
                                    op=ALU.subtract)
            if has_bvocab:
                btgt_t = p4.tile([1, ROWS], F32, tag="btgt")
                nc.sync.dma_start(btgt_t[:], d_btgt[:])
                bts_t = p4.tile([1, 1], F32, tag="bts")
                nc.vector.reduce_sum(bts_t[:], btgt_t[:], axis=AX.X)
                nc.vector.tensor_tensor(nll_t[:], nll_t[:], bts_t[:],
                                        op=ALU.subtract)
            loss_t = p4.tile([1, 1], F32, tag="loss")
            nc.vector.tensor_scalar(loss_t[:], nll_t[:], 1.0 / N, None,
                                    op0=ALU.mult)
            nc.sync.dma_start(d_loss[:], loss_t[:])
            ps4.release()
            wk4.release()
            p4.release()

        p4io.release()
        pph.release()
        ppa.release()

    nc.finalize()
    return nc


def kernel(features, captions, W_proj, b_proj, W_embed, Wx, Wh, Wattn, b,
           W_vocab, b_vocab):
    global last_exec_ns
    from concourse.bass_utils import run_bass_kernel_spmd

    features = np.asarray(features)
    captions = np.asarray(captions)
    cap_dtype = captions.dtype
    W_proj = np.asarray(W_proj, np.float32)
    b_proj = np.asarray(b_proj, np.float32)
    W_embed = np.asarray(W_embed, np.float32)
    Wx = np.asarray(Wx, np.float32)
    Wh = np.asarray(Wh, np.float32)
    Wattn = np.asarray(Wattn, np.float32)
    b = np.asarray(b, np.float32)
    W_vocab = np.asarray(W_vocab, np.float32)
    b_vocab = np.asarray(b_vocab, np.float32)

    has_b = bool(np.any(b))
    has_bvocab = bool(np.any(b_vocab))

    phases = int(os.environ.get("BASS_PHASES", "3"))
    key = (has_b, has_bvocab, phases)
    if key not in _cache:
        _cache[key] = _build(has_b, has_bvocab, phases)
    nc = _cache[key]

    cap_in = np.asarray(captions[:, :-1], np.int64)   # (N, S)
    cap_out = np.asarray(captions[:, 1:], np.int64)
    mask = (cap_out != 0).astype(np.float32)          # (N, S)
    x = W_embed[cap_in].astype(np.float32)            # (N, S, W_DIM)

    # shared (replicated) arrays
    wproj_h = np.zeros((1408, 1024), np.float32)
    wproj_h[:D_IMG] = W_proj
    wproj_h[D_IMG] = b_proj
    wproj_h = wproj_h.astype(BF)
    wh_h = Wh.astype(BF)
    wx_h = Wx.astype(BF)
    wattn_h = np.clip(Wattn * WSCALE, -240.0, 240.0).astype(FP8NP)
    wvoc_h = np.clip(W_vocab * WSCALE, -240.0, 240.0).astype(FP8NP)
    i128_h = np.eye(128, dtype=BF)
    col_n = np.arange(B * P16) // P16
    m32_h = np.where(col_n[None, :] == np.arange(B)[:, None], 0.0, NEG
                     ).astype(BF)
    bvec_h = b.reshape(1, 4096).astype(BF)
    bvoc_h = (b_vocab * (FSCALE * WSCALE)).reshape(1, V).astype(BF)

    feat = features.reshape(N, D_IMG, P16).astype(np.float32)

    in_maps = []
    for ci in range(NC):
        slc = slice(ci * B, (ci + 1) * B)
        f2t = np.zeros((1408, 512), np.float32)
        f2t[:D_IMG] = feat[slc].transpose(1, 0, 2).reshape(D_IMG, B * P16)
        f2t[D_IMG] = 1.0
        xt = x[slc].transpose(2, 1, 0).reshape(W_DIM, ROWS)  # col = 32*t + n
        tgt = cap_out[slc].T.reshape(ROWS)                   # r = 32*t + n
        mk = mask[slc].T.reshape(ROWS)
        wtgt = (W_vocab[:, tgt] * mk[None, :]).astype(BF)
        maskm = mk.reshape(MCH, MROW).T.copy()               # [row, m]
        m = {
            "f2t": f2t.astype(BF),
            "wproj": wproj_h,
            "wattn": wattn_h,
            "wh": wh_h,
            "wx": wx_h,
            "xt": xt.astype(BF),
            "wvoc": wvoc_h,
            "wtgt": wtgt,
            "maskm": maskm.astype(np.float32),
            "i128": i128_h,
            "m32": m32_h,
        }
        if has_b:
            m["bvec"] = bvec_h
        if has_bvocab:
            m["bvoc"] = bvoc_h
            m["btgt"] = (b_vocab[tgt] * mk).reshape(1, ROWS).astype(np.float32)
        in_maps.append(m)

    trace = bool(int(os.environ.get("BASS_KPROF", "0")))
    if trace:
        import sys, types
        try:
            import antenv.axon_hooks  # noqa
        except ImportError:
            import trn_agent_boot.trn_boot as _tb
            _hook = _tb._ntff_profile_via_ctypes("/opt/axon/libaxon_pjrt.so")
            _mod = types.ModuleType("antenv.axon_hooks")
            _mod.get_axon_ntff_profile_hook = lambda: _hook
            import antenv
            sys.modules["antenv.axon_hooks"] = _mod
            antenv.axon_hooks = _mod

    if os.environ.get("BASS_SIM"):
        from concourse.bass_interp import CoreSim
        sim = CoreSim(nc)
        for k2, v2 in in_maps[0].items():
            sim.tensor(k2)[:] = v2
        sim.simulate()
        print("SIM core0 partial loss:", np.asarray(sim.tensor("loss"))[0, 0],
              flush=True)
        return np.asarray(np.float32(np.asarray(sim.tensor("loss"))[0, 0] * NC))

    res = run_bass_kernel_spmd(nc, in_maps, core_ids=list(range(NC)),
                               trace=trace)
    last_exec_ns = res.exec_time_ns
    total = np.float32(0.0)
    for ci in range(NC):
        total += res.results[ci]["loss"][0, 0]
    out = np.asarray(total, np.float32)
    del cap_dtype
    return out


# revision 12
# speedup vs baseline: 2.3022x; 1.1560x over previous
"""CaptioningRNN (attention LSTM + vocab softmax loss) on 8 TRN2 NeuronCores.

Data-parallel over batch N=256 -> 32 samples/core. Weights replicated.

Performance structure:
  - Recurrence MMs emitted K-chunk-outer / unit-interleaved so consecutive
    MMs target different PE column tiles (tile_position (0,32j)) and stream
    concurrently (~4x effective MM rate).
  - fp8(e4m3) operands throughout the hot matmuls: P1/P2/P4 use DoubleRow
    (2 contraction rows/cycle), recurrence h/x parts use plain fp8 (same
    speed as bf16 but half the DMA/SBUF). Attention context stays bf16.
    Scales: h x64, A2 x16, x x1024, weights x512 (Wx x32); gate psum lands
    at x32768, scores at x1024, vocab at x32768 -- folded into activation
    scales. Target-score path stays bf16 exact.
  - Vocab score groups (4 DR MMs + exp each) are interleaved into the
    recurrence steps to fill the LSTM activation-chain stall; the rest run
    in a dense tail phase.
"""

import math
import os
import numpy as np
import ml_dtypes

BF = ml_dtypes.bfloat16
FP8NP = ml_dtypes.float8_e4m3

N, T, V, W_DIM, H, D_IMG = 256, 32, 10000, 512, 1024, 1280
P16 = 16
NC = 8
B = N // NC          # 32 samples per core
S = T - 1            # 31 steps
ROWS = B * S         # 992 (t,n) rows per core, r = 32*t + n
MCH = 8              # vocab row chunks
MROW = ROWS // MCH   # 124
VCH = 20             # vocab col chunks
VCOL = V // VCH      # 500
NEG = -1.0e9         # mask value in score psum (x1024 scale)

# fp8 scales
S_H = 64.0           # h state
S_A2 = 16.0          # A2 (attention keys)
S_X = 1024.0         # embeddings
S_W = 512.0          # Wh / Wattn / Wvocab / Wproj
S_WX = 32.0          # Wx
S_F = 16.0           # features
PS_GATE = S_H * S_W          # 32768 gate psum scale
PS_SCORE = S_H * S_A2        # 1024 score psum scale
PS_P1 = S_F * S_W            # 8192  P1 psum scale
PS_VOC = S_H * S_W           # 32768 vocab psum scale

_cache = {}

last_exec_ns = None

# vocab group scheduling: group (m, vc) usable inside step t if all its
# rows are produced by step t-1:  124*(m+1) <= 32*t
VG_TAV = {m: -(-(124 * (m + 1)) // 32) for m in range(MCH)}  # min t
VG_PER_STEP = 2


def _build(has_b, has_bvocab, phases=3):
    import concourse.mybir as mybir
    from concourse.bacc import Bacc
    from concourse.tile import TileContext

    F32 = mybir.dt.float32
    BF16 = mybir.dt.bfloat16
    FP8 = mybir.dt.float8e4
    AF = mybir.ActivationFunctionType
    ALU = mybir.AluOpType
    AX = mybir.AxisListType
    DR = mybir.MatmulPerfMode.DoubleRow

    nc = Bacc()

    d_f2t = nc.declare_dram_parameter("f2t", [1408, 512], FP8, isOutput=False)
    d_wproj = nc.declare_dram_parameter("wproj", [1408, 1024], FP8, isOutput=False)
    d_wattn = nc.declare_dram_parameter("wattn", [1024, 4096], FP8, isOutput=False)
    d_wh = nc.declare_dram_parameter("wh", [1024, 4096], FP8, isOutput=False)
    d_wx = nc.declare_dram_parameter("wx", [512, 4096], FP8, isOutput=False)
    d_xt = nc.declare_dram_parameter("xt", [512, ROWS], FP8, isOutput=False)
    d_wvoc = nc.declare_dram_parameter("wvoc", [1024, V], FP8, isOutput=False)
    d_wtgt = nc.declare_dram_parameter("wtgt", [1024, ROWS], BF16, isOutput=False)
    d_maskm = nc.declare_dram_parameter("maskm", [MROW, MCH], F32, isOutput=False)
    d_i128 = nc.declare_dram_parameter("i128", [128, 128], BF16, isOutput=False)
    d_m32 = nc.declare_dram_parameter("m32", [32, 512], BF16, isOutput=False)
    if has_b:
        d_bvec = nc.declare_dram_parameter("bvec", [1, 4096], BF16, isOutput=False)
    if has_bvocab:
        d_bvoc = nc.declare_dram_parameter("bvoc", [1, V], BF16, isOutput=False)
        d_btgt = nc.declare_dram_parameter("btgt", [1, ROWS], F32, isOutput=False)
    d_loss = nc.declare_dram_parameter("loss", [1, 1], F32, isOutput=True)

    # gate-unit tables: psA = [i0,i1,f0,f1], psB = [o0,o1,g0,g1]
    UNITS_A = [(0, 0), (0, 1), (1, 0), (1, 1)]
    UNITS_B = [(2, 0), (2, 1), (3, 0), (3, 1)]

    def loA(j):
        g, e = UNITS_A[j]
        return 1024 * g + 512 * e

    def loB(j):
        g, e = UNITS_B[j]
        return 1024 * g + 512 * e

    def sl(j):
        return slice(32 * j, 32 * (j + 1))

    with TileContext(nc) as tc:
        ppa = tc.alloc_tile_pool(name="ppa", bufs=1)
        i128_t = ppa.tile([128, 128], BF16, tag="i128")
        nc.sync.dma_start(i128_t[:], d_i128[:])
        if has_b or has_bvocab:
            ones_t = ppa.tile([1, 128], BF16, tag="ones")
            nc.vector.memset(ones_t[:], 1.0)

        pp3 = tc.alloc_tile_pool(name="pp3", bufs=1)
        ppb = tc.alloc_tile_pool(name="ppb", bufs=1)
        at8_t = pp3.tile([128, 8, 512], FP8, tag="at8")        # A2T x16
        bp_t = [pp3.tile([128, 4096], BF16, tag=f"bp{c}", name=f"bp{c}")
                for c in range(4)]                             # x PS_GATE
        h0t_t = pp3.tile([128, 8, B], BF16, tag="h0t")
        h0t8_t = pp3.tile([128, 8, B], FP8, tag="h0t8")        # x S_H
        c_t = pp3.tile([64, 512], F32, tag="cst")
        m32_t = pp3.tile([32, 512], BF16, tag="m32")
        wh_t = ppb.tile([128, 8, 4096], FP8, tag="wh")
        wx_t = ppb.tile([128, 4, 4096], FP8, tag="wx")
        xt_t = ppb.tile([128, 4, ROWS], FP8, tag="xt")

        p1 = tc.alloc_tile_pool(name="p1", bufs=1)
        p2 = tc.alloc_tile_pool(name="p2", bufs=1)
        p1ps = tc.alloc_tile_pool(name="p1ps", bufs=2, space="PSUM")
        p2ps = tc.alloc_tile_pool(name="p2ps", bufs=2, space="PSUM")

        wproj_t = p1.tile([128, 11, 1024], FP8, tag="wproj")
        f2t_t = p1.tile([128, 11, 512], FP8, tag="f2t")
        wat_t = p2.tile([128, 8, 4096], FP8, tag="wat")

        # ---- DMA issue order: P1 deps, m32, wattn, xt, wh, wx ----
        nc.sync.dma_start(
            wproj_t[:], d_wproj[:].rearrange("(c k) m -> k c m", k=128))
        nc.sync.dma_start(
            f2t_t[:], d_f2t[:].rearrange("(c k) m -> k c m", k=128))
        nc.sync.dma_start(m32_t[:], d_m32[:])
        nc.sync.dma_start(
            wat_t[:], d_wattn[:].rearrange("(c k) m -> k c m", k=128))
        nc.sync.dma_start(
            xt_t[:], d_xt[:].rearrange("(c k) m -> k c m", k=128))
        nc.sync.dma_start(
            wh_t[:], d_wh[:].rearrange("(c k) m -> k c m", k=128))
        nc.sync.dma_start(
            wx_t[:], d_wx[:].rearrange("(c k) m -> k c m", k=128))
        if has_b:
            bvec_t = pp3.tile([1, 4096], BF16, tag="bvec")
            nc.sync.dma_start(bvec_t[:], d_bvec[:])

        # ========== P1: feature projection (fp8 DR) -> at8, h0, c0 ========
        h0f_t = p1.tile([128, 8, B], F32, tag="h0f")
        for hc in range(8):
            ps = p1ps.tile([128, 512], F32, tag="pp1")
            cs = slice(128 * hc, 128 * (hc + 1))
            for u in range(5):
                nc.tensor.matmul(
                    ps[:], wproj_t[:, 2 * u:2 * u + 2, cs],
                    f2t_t[:, 2 * u:2 * u + 2, :],
                    start=(u == 0), stop=False, perf_mode=DR)
            nc.tensor.matmul(ps[:], wproj_t[:, 10, cs], f2t_t[:, 10, :],
                             start=False, stop=True)
            # psum = PS_P1 * A
            nc.vector.tensor_scalar(at8_t[:, hc, :], ps[:],
                                    S_A2 / PS_P1, None, op0=ALU.mult)
            nc.vector.reduce_sum(
                h0f_t[:, hc, :],
                ps[:].rearrange("k (n p) -> k n p", p=P16),
                axis=AX.X)
        # h0f = PS_P1 * 16 * h0
        nc.vector.tensor_scalar(h0t_t[:], h0f_t[:],
                                1.0 / (P16 * PS_P1), None, op0=ALU.mult)
        nc.vector.tensor_scalar(h0t8_t[:], h0f_t[:],
                                S_H / (P16 * PS_P1), None, op0=ALU.mult)
        c0p = p2ps.tile([64, 512], BF16, tag="c0p")
        for k in range(8):
            eta, j = k // 4, k % 4
            nc.tensor.transpose(
                c0p[32 * eta:32 * (eta + 1), 128 * j:128 * (j + 1)],
                h0t_t[:, k, :], i128_t[0:128, 0:128],
                tile_position=(0, 32 * eta))
        nc.vector.tensor_copy(c_t[:], c0p[:])

        # ========== P2: bp = A2 @ Wattn (fp8 DR), scaled to PS_GATE =======
        for nch in range(8):
            for c in range(4):
                ps = p2ps.tile([128, 512], F32, tag="pp2")
                for kc in range(4):
                    nc.tensor.matmul(
                        ps[:], at8_t[:, 2 * kc:2 * kc + 2,
                                     128 * c:128 * (c + 1)],
                        wat_t[:, 2 * kc:2 * kc + 2,
                              512 * nch:512 * (nch + 1)],
                        start=(kc == 0), stop=(kc == 3),
                        perf_mode=DR)
                # psum = (S_A2*S_W) A2@Wattn ; want PS_GATE scale
                nc.vector.tensor_scalar(
                    bp_t[c][:, 512 * nch:512 * (nch + 1)],
                    ps[:], PS_GATE / (S_A2 * S_W), None, op0=ALU.mult)

        p2ps.release()
        p1ps.release()
        p2.release()
        p1.release()

        if phases < 2:
            dbg_t = ppa.tile([1, 1], F32, tag="dbg")
            nc.vector.tensor_copy(dbg_t[:], c_t[0:1, 0:1])
            nc.sync.dma_start(d_loss[:], dbg_t[:])

        pph = tc.alloc_tile_pool(name="pph", bufs=1, side="right")
        hst_t = pph.tile([128, 8, ROWS], BF16, tag="hst")
        hst8_t = pph.tile([128, 8, ROWS], FP8, tag="hst8")     # x S_H
        se_t = pph.tile([MROW, MCH, VCH], F32, tag="SE")
        if has_bvocab:
            bvoc_all_t = pph.tile([1, VCH, VCOL], BF16, tag="bvoc")
            nc.sync.dma_start(bvoc_all_t[:],
                              d_bvoc[:].rearrange("o (c m) -> o c m", m=VCOL))
        p4io = tc.alloc_tile_pool(name="p4io", bufs=3, side="right")
        wk4 = tc.alloc_tile_pool(name="wk4", bufs=3, side="right")

        from collections import OrderedDict
        wv_ring = OrderedDict()
        wv_seq = [0]

        def get_wv(vc):
            if vc in wv_ring:
                wv_ring.move_to_end(vc)
                return wv_ring[vc]
            wv = p4io.tile([128, 8, VCOL], FP8, tag="wv",
                           name=f"wv{vc}_{wv_seq[0]}")
            wv_seq[0] += 1
            nc.sync.dma_start(
                wv[:], d_wvoc[:, VCOL * vc:VCOL * (vc + 1)]
                .rearrange("(c k) m -> k c m", k=128))
            wv_ring[vc] = wv
            while len(wv_ring) > 3:
                wv_ring.popitem(last=False)
            return wv

        def emit_vgroup(ps4v, wv, m, vc):
            ps = ps4v.tile([MROW, VCOL], F32, tag="pv")
            for kc in range(4):
                nc.tensor.matmul(
                    ps[:],
                    hst8_t[:, 2 * kc:2 * kc + 2, MROW * m:MROW * (m + 1)],
                    wv[:, 2 * kc:2 * kc + 2, :],
                    start=(kc == 0), stop=(kc == 3 and not has_bvocab),
                    perf_mode=DR)
            if has_bvocab:
                nc.tensor.matmul(
                    ps[:], ones_t[0:1, 0:MROW], bvoc_all_t[0:1, vc, :],
                    start=False, stop=True)
            return ps

        def emit_vexp(ps, m, vc):
            scr = wk4.tile([MROW, VCOL], F32, tag="scr")
            nc.scalar.activation(scr[:], ps[:], AF.Exp,
                                 scale=float(1.0 / PS_VOC),
                                 accum_out=se_t[:, m, vc:vc + 1])

        # vocab interleave schedule
        vqueue = [(m, vc) for m in range(MCH) for vc in range(VCH)]
        vptr = 0
        vdone = set()

        # ================= P3: recurrence ==========
        if phases >= 2:
            ps4v = tc.alloc_tile_pool(name="ps4v", bufs=2, space="PSUM")
            ps3 = tc.alloc_tile_pool(name="ps3", bufs=2, space="PSUM")
            psSp = tc.alloc_tile_pool(name="psSp", bufs=1, space="PSUM")
            psTp = tc.alloc_tile_pool(name="psTp", bufs=1, space="PSUM")
            wkS = tc.alloc_tile_pool(name="wkS", bufs=2)
            wkC = tc.alloc_tile_pool(name="wkC", bufs=1)

            get_wv(0)
            get_wv(1)

            def ht8(t, k):
                if t == 0:
                    return h0t8_t[:, k, :]
                return hst8_t[:, k, B * (t - 1):B * t]

            def mm(ps, lhs, rhs, start, stop, tp):
                nc.tensor.matmul(ps, lhs, rhs, start=start, stop=stop,
                                 tile_position=tp, skip_group_check=True)

            def emit_x0():
                pA = ps3.tile([128, 512], F32, tag="pA", name="pA0")
                pB = ps3.tile([128, 512], F32, tag="pB", name="pB0")
                for c in range(4):
                    for j in range(4):
                        mm(pA[sl(j), :], xt_t[:, c, 0:B],
                           wx_t[:, c, loA(j):loA(j) + 512],
                           start=(c == 0), stop=False, tp=(0, 32 * j))
                    for j in range(4):
                        mm(pB[sl(j), :], xt_t[:, c, 0:B],
                           wx_t[:, c, loB(j):loB(j) + 512],
                           start=(c == 0), stop=False, tp=(0, 32 * j))
                return pA, pB

            ps_cur = emit_x0()
            inv_s = float(1.0 / (np.sqrt(H) * PS_SCORE))
            inv_g = float(1.0 / PS_GATE)

            for t in range(S):
                psA, psB = ps_cur
                # --- scores (tile 0) interleaved with h-part of psA j1..3
                psS = psSp.tile([32, 512], F32, tag="pS")
                nc.tensor.matmul(psS[:], i128_t[0:32, 0:32], m32_t[:],
                                 start=True, stop=False)
                for k in range(8):
                    nc.tensor.matmul(psS[:], ht8(t, k), at8_t[:, k, :],
                                     start=False, stop=(k == 7))
                    for j in (1, 2, 3):
                        mm(psA[sl(j), :], ht8(t, k),
                           wh_t[:, k, loA(j):loA(j) + 512],
                           start=False, stop=False, tp=(0, 32 * j))
                # softmax (scalar + vector)
                e_t = wkS.tile([32, 512], F32, tag="e")
                se2_t = wkS.tile([32, 1], F32, tag="se2")
                nc.scalar.activation(e_t[:], psS[:], AF.Exp,
                                     scale=inv_s, accum_out=se2_t[:, 0:1])
                re_t = wkS.tile([32, 1], F32, tag="re")
                nc.vector.reciprocal(re_t[:], se2_t[:])
                w_t = wkS.tile([32, 512], BF16, tag="w")
                nc.vector.tensor_scalar(w_t[:], e_t[:], re_t[:, 0:1], None,
                                        op0=ALU.mult)
                # --- h-part psA j0 + psB j1..3
                for k in range(8):
                    mm(psA[sl(0), :], ht8(t, k),
                       wh_t[:, k, loA(0):loA(0) + 512],
                       start=False, stop=False, tp=(0, 0))
                    for j in (1, 2, 3):
                        mm(psB[sl(j), :], ht8(t, k),
                           wh_t[:, k, loB(j):loB(j) + 512],
                           start=False, stop=False, tp=(0, 32 * j))
                # --- w transposes (wait softmax)
                psW = psTp.tile([128, 4, 64], BF16, tag="pT",
                                name=f"psW{t}")[:, :, 0:32]
                for c in range(4):
                    nc.tensor.transpose(psW[:, c, :],
                                        w_t[:, 128 * c:128 * (c + 1)],
                                        i128_t[0:32, 0:32])
                wt_t = wkS.tile([128, 4, 32], BF16, tag="wt")
                nc.vector.tensor_copy(wt_t[:], psW[:])
                # --- attn psA all units (4-way)
                for c in range(4):
                    for j in range(4):
                        mm(psA[sl(j), :], wt_t[:, c, :],
                           bp_t[c][:, loA(j):loA(j) + 512],
                           start=False, stop=(c == 3 and not has_b),
                           tp=(0, 32 * j))
                if has_b:
                    for j in range(4):
                        nc.tensor.matmul(
                            psA[sl(j), :], ones_t[0:1, 0:32],
                            bvec_t[0:1, loA(j):loA(j) + 512],
                            start=False, stop=True, tile_position=(0, 32 * j),
                            skip_group_check=True)
                # sigma(i), sigma(f) + u = f*c
                si_t = wkC.tile([64, 512], F32, tag="si")
                nc.scalar.activation(si_t[:], psA[0:64, :], AF.Sigmoid,
                                     scale=inv_g)
                sf_t = wkC.tile([64, 512], F32, tag="sf")
                nc.scalar.activation(sf_t[:], psA[64:128, :], AF.Sigmoid,
                                     scale=inv_g)
                u_t = wkC.tile([64, 512], F32, tag="u")
                nc.vector.tensor_tensor(u_t[:], sf_t[:], c_t[:],
                                        op=ALU.mult)
                # --- h-part psB j0 + attn psB j1..3
                for k in range(8):
                    mm(psB[sl(0), :], ht8(t, k),
                       wh_t[:, k, loB(0):loB(0) + 512],
                       start=False, stop=False, tp=(0, 0))
                    if k < 4:
                        for j in (1, 2, 3):
                            mm(psB[sl(j), :], wt_t[:, k, :],
                               bp_t[k][:, loB(j):loB(j) + 512],
                               start=False, stop=(k == 3 and not has_b),
                               tp=(0, 32 * j))
                # --- attn psB j0 + x(t+1) psA j1..3
                if t + 1 < S:
                    pA2 = ps3.tile([128, 512], F32, tag="pA", name=f"pA{t+1}")
                    pB2 = ps3.tile([128, 512], F32, tag="pB", name=f"pB{t+1}")
                for c in range(4):
                    mm(psB[sl(0), :], wt_t[:, c, :],
                       bp_t[c][:, loB(0):loB(0) + 512],
                       start=False, stop=(c == 3 and not has_b), tp=(0, 0))
                    if t + 1 < S:
                        for j in (1, 2, 3):
                            mm(pA2[sl(j), :], xt_t[:, c, B * (t + 1):B * (t + 2)],
                               wx_t[:, c, loA(j):loA(j) + 512],
                               start=(c == 0), stop=False, tp=(0, 32 * j))
                if has_b:
                    for j in range(4):
                        nc.tensor.matmul(
                            psB[sl(j), :], ones_t[0:1, 0:32],
                            bvec_t[0:1, loB(j):loB(j) + 512],
                            start=False, stop=True, tile_position=(0, 32 * j),
                            skip_group_check=True)
                # --- gate chain + x(t+1) rest on tensor
                tg_t = wkC.tile([64, 512], F32, tag="tg")
                nc.scalar.activation(tg_t[:], psB[64:128, :], AF.Tanh,
                                     scale=inv_g)
                so_t = wkC.tile([64, 512], F32, tag="so")
                nc.scalar.activation(so_t[:], psB[0:64, :], AF.Sigmoid,
                                     scale=inv_g)
                if t + 1 < S:
                    for c in range(4):
                        mm(pA2[sl(0), :], xt_t[:, c, B * (t + 1):B * (t + 2)],
                           wx_t[:, c, loA(0):loA(0) + 512],
                           start=(c == 0), stop=False, tp=(0, 0))
                        for j in (1, 2, 3):
                            mm(pB2[sl(j), :], xt_t[:, c, B * (t + 1):B * (t + 2)],
                               wx_t[:, c, loB(j):loB(j) + 512],
                               start=(c == 0), stop=False, tp=(0, 32 * j))
                    for c in range(4):
                        mm(pB2[sl(0), :], xt_t[:, c, B * (t + 1):B * (t + 2)],
                           wx_t[:, c, loB(0):loB(0) + 512],
                           start=(c == 0), stop=False, tp=(0, 0))
                    ps_cur = (pA2, pB2)
                # --- interleaved vocab score groups (fill act-chain stall)
                vg_now = []
                budget = VG_PER_STEP
                while budget > 0 and vptr < len(vqueue) \
                        and VG_TAV[vqueue[vptr][0]] <= t:
                    m_, vc_ = vqueue[vptr]
                    wv = get_wv(vc_)
                    ps_g = emit_vgroup(ps4v, wv, m_, vc_)
                    vg_now.append((ps_g, m_, vc_))
                    vdone.add((m_, vc_))
                    vptr += 1
                    budget -= 1
                    if vptr < len(vqueue):
                        get_wv(vqueue[vptr][1])
                # --- finish gate chain
                v_t = wkC.tile([64, 512], F32, tag="v")
                nc.vector.tensor_tensor(v_t[:], si_t[:], tg_t[:],
                                        op=ALU.mult)
                nc.vector.tensor_tensor(c_t[:], u_t[:], v_t[:], op=ALU.add)
                tc_t = wkC.tile([64, 512], F32, tag="tc")
                nc.scalar.activation(tc_t[:], c_t[:], AF.Tanh)
                h_t = wkC.tile([64, 512], BF16, tag="h")
                nc.vector.tensor_tensor(h_t[:], so_t[:], tc_t[:], op=ALU.mult)
                # --- transpose h -> hst (+ fp8 copy)
                psH = psTp.tile([128, 4, 64], BF16, tag="pT",
                                name=f"psH{t}")
                for q in range(4):
                    nc.tensor.transpose(psH[:, q, :],
                                        h_t[:, 128 * q:128 * (q + 1)],
                                        i128_t[0:64, 0:64])
                dst = hst_t[:, :, B * t:B * (t + 1)].rearrange(
                    "k (eta hcp) n -> k hcp eta n", eta=2, hcp=4)
                nc.vector.tensor_copy(
                    dst, psH[:].rearrange("k hcp (eta n) -> k hcp eta n", eta=2))
                nc.vector.tensor_scalar(
                    hst8_t[:, :, B * t:B * (t + 1)],
                    hst_t[:, :, B * t:B * (t + 1)],
                    S_H, None, op0=ALU.mult)
                # vocab exps last on scalar queue
                for ps_g, m_, vc_ in vg_now:
                    emit_vexp(ps_g, m_, vc_)

            wkC.release()
            wkS.release()
            psTp.release()
            psSp.release()
            ps3.release()

        ppb.release()
        pp3.release()

        if phases == 2:
            dbg2_t = ppa.tile([1, 1], F32, tag="dbg2")
            nc.vector.tensor_copy(dbg2_t[:], c_t[0:1, 0:1])
            nc.sync.dma_start(d_loss[:], dbg2_t[:])

        # ================= P4 tail: remaining vocab groups -> loss ========
        if phases >= 3:
            p4 = tc.alloc_tile_pool(name="p4", bufs=1, side="right")

            tailq = [(vc, m) for vc in range(VCH) for m in range(MCH)
                     if (m, vc) not in vdone]
            for gi, (vc, m) in enumerate(tailq):
                wv = get_wv(vc)
                if gi + 1 < len(tailq):
                    get_wv(tailq[gi + 1][0])
                ps_g = emit_vgroup(ps4v, wv, m, vc)
                emit_vexp(ps_g, m, vc)

            # target scores: sum over all rows of hsT*WtgtT (mask folded in)
            wtgt_t = p4.tile([128, 8, ROWS], BF16, tag="wtgt")
            nc.sync.dma_start(
                wtgt_t[:], d_wtgt[:].rearrange("(c k) m -> k c m", k=128))
            tparts = p4.tile([128, 8], F32, tag="tparts")
            for k in range(8):
                scr2 = wk4.tile([128, ROWS], F32, tag="scr2")
                nc.vector.tensor_tensor(scr2[:], hst_t[:, k, :],
                                        wtgt_t[:, k, :], op=ALU.mult)
                nc.vector.reduce_sum(tparts[:, k:k + 1], scr2[:],
                                     axis=AX.X)
            tacc = p4.tile([128, 1], F32, tag="tacc")
            nc.vector.reduce_sum(tacc[:], tparts[:], axis=AX.X)
            tgt_r = p4.tile([128, 1], F32, tag="tgtr")
            import concourse.bass_isa as bass_isa
            nc.gpsimd.partition_all_reduce(tgt_r[:], tacc[:], channels=128,
                                           reduce_op=bass_isa.ReduceOp.add)

            # lse side
            ses_t = p4.tile([MROW, MCH], F32, tag="ses")
            nc.vector.reduce_sum(ses_t[:], se_t[:], axis=AX.X)
            l_t = p4.tile([MROW, MCH], F32, tag="lt")
            nc.scalar.activation(l_t[:], ses_t[:], AF.Ln)
            maskm_t = p4.tile([MROW, MCH], F32, tag="maskm")
            nc.sync.dma_start(maskm_t[:], d_maskm[:])
            lm_t = p4.tile([MROW, MCH], F32, tag="lm")
            nc.vector.tensor_tensor(lm_t[:], l_t[:], maskm_t[:], op=ALU.mult)
            lr_t = p4.tile([MROW, 1], F32, tag="lr")
            nc.vector.reduce_sum(lr_t[:], lm_t[:], axis=AX.X)
            lse_r = p4.tile([MROW, 1], F32, tag="lser")
            nc.gpsimd.partition_all_reduce(lse_r[:], lr_t[:], channels=MROW,
                                           reduce_op=bass_isa.ReduceOp.add)

            nll_t = p4.tile([1, 1], F32, tag="nll")
            nc.vector.tensor_tensor(nll_t[:], lse_r[0:1, :], tgt_r[0:1, :],
                                    op=ALU.subtract)
            if has_bvocab:
                btgt_t = p4.tile([1, ROWS], F32, tag="btgt")
                nc.sync.dma_start(btgt_t[:], d_btgt[:])
                bts_t = p4.tile([1, 1], F32, tag="bts")
                nc.vector.reduce_sum(bts_t[:], btgt_t[:], axis=AX.X)
                nc.vector.tensor_tensor(nll_t[:], nll_t[:], bts_t[:],
                                        op=ALU.subtract)
            loss_t = p4.tile([1, 1], F32, tag="loss")
            nc.vector.tensor_scalar(loss_t[:], nll_t[:], 1.0 / N, None,
                                    op0=ALU.mult)
            nc.sync.dma_start(d_loss[:], loss_t[:])
            p4.release()

        if phases >= 2:
            ps4v.release()
        wk4.release()
        p4io.release()
        pph.release()
        ppa.release()

    nc.finalize()
    return nc


def kernel(features, captions, W_proj, b_proj, W_embed, Wx, Wh, Wattn, b,
           W_vocab, b_vocab):
    global last_exec_ns
    from concourse.bass_utils import run_bass_kernel_spmd

    features = np.asarray(features)
    captions = np.asarray(captions)
    cap_dtype = captions.dtype
    W_proj = np.asarray(W_proj, np.float32)
    b_proj = np.asarray(b_proj, np.float32)
    W_embed = np.asarray(W_embed, np.float32)
    Wx = np.asarray(Wx, np.float32)
    Wh = np.asarray(Wh, np.float32)
    Wattn = np.asarray(Wattn, np.float32)
    b = np.asarray(b, np.float32)
    W_vocab = np.asarray(W_vocab, np.float32)
    b_vocab = np.asarray(b_vocab, np.float32)

    has_b = bool(np.any(b))
    has_bvocab = bool(np.any(b_vocab))

    phases = int(os.environ.get("BASS_PHASES", "3"))
    key = (has_b, has_bvocab, phases)
    if key not in _cache:
        _cache[key] = _build(has_b, has_bvocab, phases)
    nc = _cache[key]

    cap_in = np.asarray(captions[:, :-1], np.int64)   # (N, S)
    cap_out = np.asarray(captions[:, 1:], np.int64)
    mask = (cap_out != 0).astype(np.float32)          # (N, S)
    x = W_embed[cap_in].astype(np.float32)            # (N, S, W_DIM)

    def q8(a, s):
        return np.clip(a * s, -240.0, 240.0).astype(FP8NP)

    # shared (replicated) arrays
    wproj_h = np.zeros((1408, 1024), np.float32)
    wproj_h[:D_IMG] = W_proj
    # bias row: pairs with f2t row of S_F -> contributes S_F*S_W*b = PS_P1*b
    wproj_h[D_IMG] = b_proj
    wproj_h = q8(wproj_h, S_W)
    wh_h = q8(Wh, S_W)
    wx_h = q8(Wx, S_WX)
    wattn_h = q8(Wattn, S_W)
    wvoc_h = q8(W_vocab, S_W)
    i128_h = np.eye(128, dtype=BF)
    col_n = np.arange(B * P16) // P16
    m32_h = np.where(col_n[None, :] == np.arange(B)[:, None], 0.0, NEG
                     ).astype(BF)
    bvec_h = (b * PS_GATE).reshape(1, 4096).astype(BF)
    bvoc_h = (b_vocab * PS_VOC).reshape(1, V).astype(BF)

    feat = features.reshape(N, D_IMG, P16).astype(np.float32)

    in_maps = []
    for ci in range(NC):
        slc = slice(ci * B, (ci + 1) * B)
        f2t = np.zeros((1408, 512), np.float32)
        f2t[:D_IMG] = feat[slc].transpose(1, 0, 2).reshape(D_IMG, B * P16)
        f2t[D_IMG] = 1.0
        xt = x[slc].transpose(2, 1, 0).reshape(W_DIM, ROWS)  # col = 32*t + n
        tgt = cap_out[slc].T.reshape(ROWS)                   # r = 32*t + n
        mk = mask[slc].T.reshape(ROWS)
        wtgt = (W_vocab[:, tgt] * mk[None, :]).astype(BF)
        maskm = mk.reshape(MCH, MROW).T.copy()               # [row, m]
        m = {
            "f2t": q8(f2t, S_F),
            "wproj": wproj_h,
            "wattn": wattn_h,
            "wh": wh_h,
            "wx": wx_h,
            "xt": q8(xt, S_X),
            "wvoc": wvoc_h,
            "wtgt": wtgt,
            "maskm": maskm.astype(np.float32),
            "i128": i128_h,
            "m32": m32_h,
        }
        if has_b:
            m["bvec"] = bvec_h
        if has_bvocab:
            m["bvoc"] = bvoc_h
            m["btgt"] = (b_vocab[tgt] * mk).reshape(1, ROWS).astype(np.float32)
        in_maps.append(m)

    trace = bool(int(os.environ.get("BASS_KPROF", "0")))
    if trace:
        import sys, types
        try:
            import antenv.axon_hooks  # noqa
        except ImportError:
            import trn_agent_boot.trn_boot as _tb
            _hook = _tb._ntff_profile_via_ctypes("/opt/axon/libaxon_pjrt.so")
            _mod = types.ModuleType("antenv.axon_hooks")
            _mod.get_axon_ntff_profile_hook = lambda: _hook
            import antenv
            sys.modules["antenv.axon_hooks"] = _mod
            antenv.axon_hooks = _mod

    if os.environ.get("BASS_SIM"):
        from concourse.bass_interp import CoreSim
        sim = CoreSim(nc)
        for k2, v2 in in_maps[0].items():
            sim.tensor(k2)[:] = v2
        sim.simulate()
        print("SIM core0 partial loss:", np.asarray(sim.tensor("loss"))[0, 0],
              flush=True)
        return np.asarray(np.float32(np.asarray(sim.tensor("loss"))[0, 0] * NC))

    res = run_bass_kernel_spmd(nc, in_maps, core_ids=list(range(NC)),
                               trace=trace)
    last_exec_ns = res.exec_time_ns
    total = np.float32(0.0)
    for ci in range(NC):
        total += res.results[ci]["loss"][0, 0]
    out = np.asarray(total, np.float32)
    del cap_dtype
    return out
